# revision 9
# baseline (speedup 1.0000x reference)
"""Trainium2 Bass kernel for a 2-layer GCN (GRACE encoder) on 8 NeuronCores.

Math (per layer, from the reference):
    h   = Z @ W
    deg = bincount(dst)            (self-loops included in edge list)
    dinv = deg^-1/2
    out = PReLU(segment_sum(h[src] * dinv[src] * dinv[dst], dst) + b)

We use dinv[s]*h[s] = ((dinv*Z) @ W)[s] =: P[s], so the per-edge work is a
pure row-gather of P plus a segment-sum, and all scaling is per-node:
    out = PReLU(dinv * segment_sum(P[src], dst) + b)

Sharding: dst-partitioned. Core c owns dst rows [c*12544, (c+1)*12544).
Each core computes P for its own rows, an AllGather makes the full P table
visible everywhere, and the scatter (segment-sum) is done with one-hot
selection matmuls accumulating in PSUM, 128 edges per matmul.
"""

import sys

for p in ("/opt/trn_rl_repo", "/opt/trn_rl_repo/concourse"):
    if p not in sys.path:
        sys.path.insert(0, p)

import numpy as np

import concourse.bass as bass
import concourse.bacc as bacc
import concourse.tile as tile
from concourse import mybir
from concourse.bass_utils import run_bass_kernel_spmd
from concourse.masks import make_identity

N = 100000
E = 1600000
FIN = 128
HID = 128
FOUT = 64
NCORES = 8
BPC = 12544          # dst rows per core (padded); 8 * 12544 = 100352
NPAD = NCORES * BPC
NBLK = BPC // 128    # 98 dst blocks of 128 per core
PCH = 128            # edges per matmul chunk

# dtype for the P tables / messages / selection matrices / weights
TABLE_DT = mybir.dt.float32
TABLE_NP = mybir.dt.np(TABLE_DT)

_cache = {}


def _preprocess(edge_index):
    """Sort edges by (dst block, src), pad per-block chunk counts uniformly
    across cores. Returns dinv, per-core index arrays, and chunk layout."""
    src = np.concatenate([edge_index[0], np.arange(N, dtype=np.int32)])
    dst = np.concatenate([edge_index[1], np.arange(N, dtype=np.int32)])
    deg = np.bincount(dst, minlength=N).astype(np.float32)
    dinv = np.zeros(NPAD, np.float32)
    dinv[:N] = np.where(deg > 0, 1.0 / np.sqrt(deg), 0.0)

    blk = dst >> 7                      # global 128-row dst block id
    order = np.lexsort((src, blk))      # block-major, ascending src inside
    src_s = src[order].astype(np.int32)
    dst_s = dst[order].astype(np.int32)
    blk_s = blk[order]

    nblk_glob = NPAD // 128             # 784
    counts = np.bincount(blk_s, minlength=nblk_glob)
    # chunks needed per local block index, maxed across cores (SPMD shape)
    Kj = np.ceil(counts.reshape(NCORES, NBLK) / PCH).astype(np.int64).max(axis=0)
    Kj = np.maximum(Kj, 1)
    off = np.zeros(NBLK, np.int64)
    off[1:] = np.cumsum(Kj)[:-1]
    C = int(Kj.sum())

    bstart = np.zeros(nblk_glob + 1, np.int64)
    bstart[1:] = np.cumsum(counts)

    srcs_dev = np.empty((NCORES, 128, C), np.int32)
    ldst_dev = np.empty((NCORES, 128, C), TABLE_NP)
    for c in range(NCORES):
        sa = np.zeros(C * PCH, np.int32)
        la = np.full(C * PCH, 255.0, np.float32)
        for j in range(NBLK):
            g = c * NBLK + j
            s0, s1 = bstart[g], bstart[g + 1]
            n = int(s1 - s0)
            pos = int(off[j]) * PCH
            sa[pos:pos + n] = src_s[s0:s1]
            la[pos:pos + n] = (dst_s[s0:s1] - (g << 7)).astype(np.float32)
        srcs_dev[c] = sa.reshape(C, PCH).T
        ldst_dev[c] = la.reshape(C, PCH).T.astype(TABLE_NP)

    return dinv, srcs_dev, ldst_dev, tuple(int(k) for k in Kj), C


def _build(Kj, C, a_val):
    """Build the SPMD Bass program (identical on all cores)."""
    nc = bacc.Bacc("TRN2", target_bir_lowering=False, debug=False,
                   num_devices=NCORES)
    DT = TABLE_DT
    f32 = mybir.dt.float32

    xT = nc.dram_tensor("xT", [128, BPC], DT, kind="ExternalInput")
    srcs = nc.dram_tensor("srcs", [128, C], mybir.dt.int32, kind="ExternalInput")
    ldst = nc.dram_tensor("ldst", [128, C], DT, kind="ExternalInput")
    W1 = nc.dram_tensor("W1", [FIN, HID], DT, kind="ExternalInput")
    W2 = nc.dram_tensor("W2", [HID, FOUT], DT, kind="ExternalInput")
    b1 = nc.dram_tensor("b1", [128, HID], f32, kind="ExternalInput")
    b2 = nc.dram_tensor("b2", [128, FOUT], f32, kind="ExternalInput")
    dinvb = nc.dram_tensor("dinvb", [128, NBLK], f32, kind="ExternalInput")
    iota = nc.dram_tensor("iota", [128, 128], DT, kind="ExternalInput")
    out = nc.dram_tensor("out", [BPC, FOUT], f32, kind="ExternalOutput")

    P1_my = nc.dram_tensor("P1_my", [BPC, HID], DT, kind="Internal")
    P1_full = nc.dram_tensor("P1_full", [NPAD, HID], DT, kind="Internal")
    P2_my = nc.dram_tensor("P2_my", [BPC, FOUT], DT, kind="Internal")
    P2_full = nc.dram_tensor("P2_full", [NPAD, FOUT], DT, kind="Internal")

    off = [0] * NBLK
    for j in range(1, NBLK):
        off[j] = off[j - 1] + Kj[j - 1]
    KMAX = max(Kj)

    with tile.TileContext(nc) as tc:
        with (
            tc.tile_pool(name="persist", bufs=1) as pp,
            tc.tile_pool(name="work", bufs=4) as wp,
            tc.tile_pool(name="gath", bufs=8) as gp,
            tc.tile_pool(name="psA", bufs=2, space="PSUM") as psA,
            tc.tile_pool(name="psB", bufs=2, space="PSUM") as psB,
        ):
            # ---- persistent SBUF state ----
            xT_sb = pp.tile([128, BPC], DT)
            nc.sync.dma_start(out=xT_sb[:], in_=xT[:])
            srcs_sb = pp.tile([128, C], mybir.dt.int32)
            nc.sync.dma_start(out=srcs_sb[:], in_=srcs[:])
            ldst_sb = pp.tile([128, C], DT)
            nc.sync.dma_start(out=ldst_sb[:], in_=ldst[:])
            W1_sb = pp.tile([FIN, HID], DT)
            nc.sync.dma_start(out=W1_sb[:], in_=W1[:])
            W2_sb = pp.tile([HID, FOUT], DT)
            nc.sync.dma_start(out=W2_sb[:], in_=W2[:])
            b1_sb = pp.tile([128, HID], f32)
            nc.sync.dma_start(out=b1_sb[:], in_=b1[:])
            b2_sb = pp.tile([128, FOUT], f32)
            nc.sync.dma_start(out=b2_sb[:], in_=b2[:])
            dinv_sb = pp.tile([128, NBLK], f32)
            nc.sync.dma_start(out=dinv_sb[:], in_=dinvb[:])
            iota_sb = pp.tile([128, 128], DT)
            nc.sync.dma_start(out=iota_sb[:], in_=iota[:])
            ident_sb = pp.tile([128, 128], DT)
            make_identity(nc, ident_sb[:])
            h1T_sb = pp.tile([128, BPC], DT)   # transposed layer-1 output

            # ---- phase A: P1 = dinv * (x @ W1), own shard ----
            for j in range(NBLK):
                ps = psA.tile([128, HID], f32, tag="pcomp")
                nc.tensor.matmul(out=ps[:], lhsT=xT_sb[:, j * 128:(j + 1) * 128],
                                 rhs=W1_sb[:], start=True, stop=True)
                p1t = wp.tile([128, HID], DT, tag="ptile")
                nc.vector.tensor_scalar_mul(p1t[:], ps[:], dinv_sb[:, j:j + 1])
                nc.sync.dma_start(out=P1_my[j * 128:(j + 1) * 128, :], in_=p1t[:])

            # ---- all-gather P1 shards -> full table ----
            nc.gpsimd.collective_compute(
                "AllGather", mybir.AluOpType.bypass,
                replica_groups=[list(range(NCORES))],
                ins=[P1_my[:]], outs=[P1_full[:]],
            )

            # ---- phase B: layer-1 gather + scatter matmuls ----
            for j in range(NBLK):
                k = Kj[j]
                o = off[j]
                agg = psA.tile([128, HID], f32, tag="agg")
                for q in range(k):
                    msg = gp.tile([128, HID], DT, tag="msg1")
                    nc.gpsimd.indirect_dma_start(
                        out=msg[:], out_offset=None,
                        in_=P1_full[:],
                        in_offset=bass.IndirectOffsetOnAxis(
                            ap=srcs_sb[:, o + q:o + q + 1], axis=0),
                    )
                    sel = wp.tile([128, 128], DT, tag="sel")
                    nc.vector.tensor_tensor(
                        out=sel[:],
                        in0=ldst_sb[:, o + q:o + q + 1].to_broadcast([128, 128]),
                        in1=iota_sb[:], op=mybir.AluOpType.is_equal)
                    nc.tensor.matmul(out=agg[:], lhsT=sel[:],
                                     rhs=msg[:],
                                     start=(q == 0), stop=(q == k - 1))
                # finalize: h1 = PReLU(dinv*agg + b1)
                z = wp.tile([128, HID], f32, tag="z1")
                nc.vector.tensor_scalar_mul(z[:], agg[:], dinv_sb[:, j:j + 1])
                nc.vector.tensor_tensor(out=z[:], in0=z[:], in1=b1_sb[:],
                                        op=mybir.AluOpType.add)
                za = wp.tile([128, HID], f32, tag="za1")
                nc.vector.tensor_scalar_mul(za[:], z[:], float(a_val))
                h1 = wp.tile([128, HID], DT, tag="h1")
                nc.vector.tensor_tensor(out=h1[:], in0=z[:], in1=za[:],
                                        op=mybir.AluOpType.max)
                # transpose for the layer-2 P matmul
                pt = psB.tile([128, 128], DT, tag="tpose")
                nc.tensor.transpose(out=pt[:], in_=h1[:], identity=ident_sb[:])
                nc.vector.tensor_copy(h1T_sb[:, j * 128:(j + 1) * 128], pt[:])

            # ---- phase C: P2 = dinv * (h1 @ W2), own shard ----
            for j in range(NBLK):
                ps = psA.tile([128, FOUT], f32, tag="pcomp")
                nc.tensor.matmul(out=ps[:], lhsT=h1T_sb[:, j * 128:(j + 1) * 128],
                                 rhs=W2_sb[:], start=True, stop=True)
                p2t = wp.tile([128, FOUT], DT, tag="ptile")
                nc.vector.tensor_scalar_mul(p2t[:], ps[:], dinv_sb[:, j:j + 1])
                nc.sync.dma_start(out=P2_my[j * 128:(j + 1) * 128, :], in_=p2t[:])

            nc.gpsimd.collective_compute(
                "AllGather", mybir.AluOpType.bypass,
                replica_groups=[list(range(NCORES))],
                ins=[P2_my[:]], outs=[P2_full[:]],
            )

            # ---- phase D: layer-2 gather + scatter + finalize ----
            for j in range(NBLK):
                k = Kj[j]
                o = off[j]
                agg = psA.tile([128, FOUT], f32, tag="agg")
                for q in range(k):
                    msg = gp.tile([128, FOUT], DT, tag="msg2")
                    nc.gpsimd.indirect_dma_start(
                        out=msg[:], out_offset=None,
                        in_=P2_full[:],
                        in_offset=bass.IndirectOffsetOnAxis(
                            ap=srcs_sb[:, o + q:o + q + 1], axis=0),
                    )
                    sel = wp.tile([128, 128], DT, tag="sel")
                    nc.vector.tensor_tensor(
                        out=sel[:],
                        in0=ldst_sb[:, o + q:o + q + 1].to_broadcast([128, 128]),
                        in1=iota_sb[:], op=mybir.AluOpType.is_equal)
                    nc.tensor.matmul(out=agg[:], lhsT=sel[:],
                                     rhs=msg[:],
                                     start=(q == 0), stop=(q == k - 1))
                z = wp.tile([128, FOUT], f32, tag="z2")
                nc.vector.tensor_scalar_mul(z[:], agg[:], dinv_sb[:, j:j + 1])
                nc.vector.tensor_tensor(out=z[:], in0=z[:], in1=b2_sb[:],
                                        op=mybir.AluOpType.add)
                za = wp.tile([128, FOUT], f32, tag="za2")
                nc.vector.tensor_scalar_mul(za[:], z[:], float(a_val))
                yo = wp.tile([128, FOUT], f32, tag="yo")
                nc.vector.tensor_tensor(out=yo[:], in0=z[:], in1=za[:],
                                        op=mybir.AluOpType.max)
                nc.sync.dma_start(out=out[j * 128:(j + 1) * 128, :], in_=yo[:])

    nc.compile()
    return nc


def _stage_inputs(x, W1, b1, W2, b2, dinv, srcs_dev, ldst_dev):
    x_pad = np.zeros((NPAD, FIN), TABLE_NP)
    x_pad[:N] = x
    in_maps = []
    W1d = W1.astype(TABLE_NP)
    W2d = W2.astype(TABLE_NP)
    b1d = np.broadcast_to(b1, (128, HID)).astype(np.float32).copy()
    b2d = np.broadcast_to(b2, (128, FOUT)).astype(np.float32).copy()
    iota_np = np.tile(np.arange(128, dtype=TABLE_NP), (128, 1)).copy()
    for c in range(NCORES):
        lo, hi = c * BPC, (c + 1) * BPC
        in_maps.append({
            "xT": np.ascontiguousarray(x_pad[lo:hi].T),
            "srcs": np.ascontiguousarray(srcs_dev[c]),
            "ldst": np.ascontiguousarray(ldst_dev[c]),
            "W1": W1d, "W2": W2d, "b1": b1d, "b2": b2d,
            "dinvb": np.ascontiguousarray(dinv[lo:hi].reshape(NBLK, 128).T),
            "iota": iota_np,
        })
    return in_maps


def kernel(x, edge_index, W1, b1, W2, b2, a, _want_results=False, _trace=False):
    x = np.asarray(x, np.float32)
    edge_index = np.asarray(edge_index, np.int32)
    dinv, srcs_dev, ldst_dev, Kj, C = _preprocess(edge_index)
    key = (Kj, float(a))
    if key not in _cache:
        _cache[key] = _build(Kj, C, float(a))
    nc = _cache[key]
    in_maps = _stage_inputs(x, np.asarray(W1, np.float32), np.asarray(b1, np.float32),
                            np.asarray(W2, np.float32), np.asarray(b2, np.float32),
                            dinv, srcs_dev, ldst_dev)
    res = run_bass_kernel_spmd(nc, in_maps, core_ids=list(range(NCORES)),
                               trace=_trace)
    outs = [res.results[c]["out"] for c in range(NCORES)]
    full = np.concatenate(outs, axis=0)[:N]
    if _want_results:
        return full.astype(np.float32), res
    return full.astype(np.float32)


# revision 16
# speedup vs baseline: 1.1300x; 1.1300x over previous
"""Trainium2 Bass kernel for a 2-layer GCN (GRACE encoder) on 8 NeuronCores.

Math (per layer, from the reference):
    h   = Z @ W
    deg = bincount(dst)            (self-loops included in edge list)
    dinv = deg^-1/2
    out = PReLU(segment_sum(h[src] * dinv[src] * dinv[dst], dst) + b)

We use dinv[s]*h[s] = ((dinv*Z) @ W)[s] =: P[s], so the per-edge work is a
pure row-gather of P plus a segment-sum, and all scaling is per-node:
    out = PReLU(dinv * segment_sum(P[src], dst) + b)

Sharding: dst-partitioned. Core c owns dst rows [c*12544, (c+1)*12544).
Each core computes P for its own rows, an AllGather makes the full P table
visible everywhere, and the scatter (segment-sum) is done with one-hot
selection matmuls accumulating in PSUM, 128 edges per matmul.
"""

import sys

for p in ("/opt/trn_rl_repo", "/opt/trn_rl_repo/concourse"):
    if p not in sys.path:
        sys.path.insert(0, p)

import numpy as np

import concourse.bass as bass
import concourse.bacc as bacc
import concourse.tile as tile
from concourse import mybir
from concourse.bass_utils import run_bass_kernel_spmd
from concourse.masks import make_identity

N = 100000
E = 1600000
FIN = 128
HID = 128
FOUT = 64
NCORES = 8
BPC = 12544          # dst rows per core (padded); 8 * 12544 = 100352
NPAD = NCORES * BPC
NBLK = BPC // 128    # 98 dst blocks of 128 per core
PCH = 128            # edges per matmul chunk

# dtype for the P tables / messages / selection matrices / weights
TABLE_DT = mybir.dt.float32
TABLE_NP = mybir.dt.np(TABLE_DT)

_cache = {}


def _preprocess(edge_index):
    """Sort edges by (dst block, src), pad per-block chunk counts uniformly
    across cores. Returns dinv, per-core index arrays, and chunk layout."""
    src = np.concatenate([edge_index[0], np.arange(N, dtype=np.int32)])
    dst = np.concatenate([edge_index[1], np.arange(N, dtype=np.int32)])
    deg = np.bincount(dst, minlength=N).astype(np.float32)
    dinv = np.zeros(NPAD, np.float32)
    dinv[:N] = np.where(deg > 0, 1.0 / np.sqrt(deg), 0.0)

    blk = dst >> 7                      # global 128-row dst block id
    order = np.lexsort((src, blk))      # block-major, ascending src inside
    src_s = src[order].astype(np.int32)
    dst_s = dst[order].astype(np.int32)
    blk_s = blk[order]

    nblk_glob = NPAD // 128             # 784
    counts = np.bincount(blk_s, minlength=nblk_glob)
    # chunks needed per local block index, maxed across cores (SPMD shape)
    Kj = np.ceil(counts.reshape(NCORES, NBLK) / PCH).astype(np.int64).max(axis=0)
    Kj = np.maximum(Kj, 1)
    off = np.zeros(NBLK, np.int64)
    off[1:] = np.cumsum(Kj)[:-1]
    C = int(Kj.sum())

    bstart = np.zeros(nblk_glob + 1, np.int64)
    bstart[1:] = np.cumsum(counts)

    srcs_dev = np.empty((NCORES, 128, C), np.int32)
    ldst_dev = np.empty((NCORES, 128, C), TABLE_NP)
    for c in range(NCORES):
        sa = np.zeros(C * PCH, np.int32)
        la = np.full(C * PCH, 255.0, np.float32)
        for j in range(NBLK):
            g = c * NBLK + j
            s0, s1 = bstart[g], bstart[g + 1]
            n = int(s1 - s0)
            pos = int(off[j]) * PCH
            sa[pos:pos + n] = src_s[s0:s1]
            la[pos:pos + n] = (dst_s[s0:s1] - (g << 7)).astype(np.float32)
        srcs_dev[c] = sa.reshape(C, PCH).T
        ldst_dev[c] = la.reshape(C, PCH).T.astype(TABLE_NP)

    return dinv, srcs_dev, ldst_dev, tuple(int(k) for k in Kj), C


def _build(Kj, C, a_val, gather_mode="indirect", do_collective=True):
    """Build the SPMD Bass program (identical on all cores).

    gather_mode/do_collective are timing-attribution knobs (defaults = real
    kernel); "linear" replaces gathers with same-size sequential reads and
    do_collective=False skips the AllGathers — both produce wrong outputs.
    """
    nc = bacc.Bacc("TRN2", target_bir_lowering=False, debug=False,
                   num_devices=NCORES)
    DT = TABLE_DT
    f32 = mybir.dt.float32

    xT = nc.dram_tensor("xT", [128, BPC], DT, kind="ExternalInput")
    srcs = nc.dram_tensor("srcs", [128, C], mybir.dt.int32, kind="ExternalInput")
    ldst = nc.dram_tensor("ldst", [128, C], DT, kind="ExternalInput")
    W1 = nc.dram_tensor("W1", [FIN, HID], DT, kind="ExternalInput")
    W2 = nc.dram_tensor("W2", [HID, FOUT], DT, kind="ExternalInput")
    b1 = nc.dram_tensor("b1", [128, HID], f32, kind="ExternalInput")
    b2 = nc.dram_tensor("b2", [128, FOUT], f32, kind="ExternalInput")
    dinvb = nc.dram_tensor("dinvb", [128, NBLK], f32, kind="ExternalInput")
    iota = nc.dram_tensor("iota", [128, 128], DT, kind="ExternalInput")
    out = nc.dram_tensor("out", [BPC, FOUT], f32, kind="ExternalOutput")

    P1_my = nc.dram_tensor("P1_my", [BPC, HID], DT, kind="Internal")
    P1_full = nc.dram_tensor("P1_full", [NPAD, HID], DT, kind="Internal")
    P2_my = nc.dram_tensor("P2_my", [BPC, FOUT], DT, kind="Internal")
    P2_full = nc.dram_tensor("P2_full", [NPAD, FOUT], DT, kind="Internal")

    off = [0] * NBLK
    for j in range(1, NBLK):
        off[j] = off[j - 1] + Kj[j - 1]
    KMAX = max(Kj)

    with tile.TileContext(nc) as tc:
        with (
            tc.tile_pool(name="persist", bufs=1) as pp,
            tc.tile_pool(name="work", bufs=4) as wp,
            tc.tile_pool(name="gath", bufs=8) as gp,
            tc.tile_pool(name="psA", bufs=2, space="PSUM") as psA,
            tc.tile_pool(name="psB", bufs=2, space="PSUM") as psB,
        ):
            # ---- persistent SBUF state ----
            xT_sb = pp.tile([128, BPC], DT)
            nc.sync.dma_start(out=xT_sb[:], in_=xT[:])
            srcs_sb = pp.tile([128, C], mybir.dt.int32)
            nc.sync.dma_start(out=srcs_sb[:], in_=srcs[:])
            ldst_sb = pp.tile([128, C], DT)
            nc.sync.dma_start(out=ldst_sb[:], in_=ldst[:])
            W1_sb = pp.tile([FIN, HID], DT)
            nc.sync.dma_start(out=W1_sb[:], in_=W1[:])
            W2_sb = pp.tile([HID, FOUT], DT)
            nc.sync.dma_start(out=W2_sb[:], in_=W2[:])
            b1_sb = pp.tile([128, HID], f32)
            nc.sync.dma_start(out=b1_sb[:], in_=b1[:])
            b2_sb = pp.tile([128, FOUT], f32)
            nc.sync.dma_start(out=b2_sb[:], in_=b2[:])
            dinv_sb = pp.tile([128, NBLK], f32)
            nc.sync.dma_start(out=dinv_sb[:], in_=dinvb[:])
            iota_sb = pp.tile([128, 128], DT)
            nc.sync.dma_start(out=iota_sb[:], in_=iota[:])
            ident_sb = pp.tile([128, 128], DT)
            make_identity(nc, ident_sb[:])
            h1T_sb = pp.tile([128, BPC], DT)   # transposed layer-1 output

            # ---- phase A: P1 = dinv * (x @ W1), own shard ----
            for j in range(NBLK):
                ps = psA.tile([128, HID], f32, tag="pcomp")
                nc.tensor.matmul(out=ps[:], lhsT=xT_sb[:, j * 128:(j + 1) * 128],
                                 rhs=W1_sb[:], start=True, stop=True)
                p1t = wp.tile([128, HID], DT, tag="ptile")
                nc.vector.tensor_scalar_mul(p1t[:], ps[:], dinv_sb[:, j:j + 1])
                nc.sync.dma_start(out=P1_my[j * 128:(j + 1) * 128, :], in_=p1t[:])

            # ---- all-gather P1 shards -> full table ----
            if do_collective:
                nc.gpsimd.collective_compute(
                    "AllGather", mybir.AluOpType.bypass,
                    replica_groups=[list(range(NCORES))],
                    ins=[P1_my[:]], outs=[P1_full[:]],
                )
            else:
                nc.sync.dma_start(out=P1_full[:BPC, :], in_=P1_my[:])

            # ---- phase B: layer-1 gather + scatter matmuls ----
            for j in range(NBLK):
                k = Kj[j]
                o = off[j]
                agg = psA.tile([128, HID], f32, tag="agg")
                selg = wp.tile([128, KMAX * 128], DT, tag="selg")
                nc.vector.tensor_tensor(
                    out=selg[:, :k * 128].rearrange("p (a b) -> p a b", a=k),
                    in0=ldst_sb[:, o:o + k, None].to_broadcast([128, k, 128]),
                    in1=iota_sb[:, None, :].to_broadcast([128, k, 128]),
                    op=mybir.AluOpType.is_equal)
                for q in range(k):
                    msg = gp.tile([128, HID], DT, tag="msg1")
                    if gather_mode == "indirect":
                        nc.gpsimd.indirect_dma_start(
                            out=msg[:], out_offset=None,
                            in_=P1_full[:],
                            in_offset=bass.IndirectOffsetOnAxis(
                                ap=srcs_sb[:, o + q:o + q + 1], axis=0),
                        )
                    else:
                        r = (j * 128) % (NPAD - 128)
                        nc.sync.dma_start(out=msg[:], in_=P1_full[r:r + 128, :])
                    nc.tensor.matmul(out=agg[:], lhsT=selg[:, q * 128:(q + 1) * 128],
                                     rhs=msg[:],
                                     start=(q == 0), stop=(q == k - 1))
                # finalize: h1 = PReLU(dinv*agg + b1)
                z = wp.tile([128, HID], f32, tag="z1")
                nc.vector.tensor_scalar_mul(z[:], agg[:], dinv_sb[:, j:j + 1])
                nc.vector.tensor_tensor(out=z[:], in0=z[:], in1=b1_sb[:],
                                        op=mybir.AluOpType.add)
                za = wp.tile([128, HID], f32, tag="za1")
                nc.vector.tensor_scalar_mul(za[:], z[:], float(a_val))
                h1 = wp.tile([128, HID], DT, tag="h1")
                nc.vector.tensor_tensor(out=h1[:], in0=z[:], in1=za[:],
                                        op=mybir.AluOpType.max)
                # transpose for the layer-2 P matmul
                pt = psB.tile([128, 128], DT, tag="tpose")
                nc.tensor.transpose(out=pt[:], in_=h1[:], identity=ident_sb[:])
                nc.vector.tensor_copy(h1T_sb[:, j * 128:(j + 1) * 128], pt[:])

            # ---- phase C: P2 = dinv * (h1 @ W2), own shard ----
            for j in range(NBLK):
                ps = psA.tile([128, FOUT], f32, tag="pcomp")
                nc.tensor.matmul(out=ps[:], lhsT=h1T_sb[:, j * 128:(j + 1) * 128],
                                 rhs=W2_sb[:], start=True, stop=True)
                p2t = wp.tile([128, FOUT], DT, tag="ptile")
                nc.vector.tensor_scalar_mul(p2t[:], ps[:], dinv_sb[:, j:j + 1])
                nc.sync.dma_start(out=P2_my[j * 128:(j + 1) * 128, :], in_=p2t[:])

            if do_collective:
                nc.gpsimd.collective_compute(
                    "AllGather", mybir.AluOpType.bypass,
                    replica_groups=[list(range(NCORES))],
                    ins=[P2_my[:]], outs=[P2_full[:]],
                )
            else:
                nc.sync.dma_start(out=P2_full[:BPC, :], in_=P2_my[:])

            # ---- phase D: layer-2 gather + scatter + finalize ----
            for j in range(NBLK):
                k = Kj[j]
                o = off[j]
                agg = psA.tile([128, FOUT], f32, tag="agg")
                selg = wp.tile([128, KMAX * 128], DT, tag="selg")
                nc.vector.tensor_tensor(
                    out=selg[:, :k * 128].rearrange("p (a b) -> p a b", a=k),
                    in0=ldst_sb[:, o:o + k, None].to_broadcast([128, k, 128]),
                    in1=iota_sb[:, None, :].to_broadcast([128, k, 128]),
                    op=mybir.AluOpType.is_equal)
                for q in range(k):
                    msg = gp.tile([128, FOUT], DT, tag="msg2")
                    if gather_mode == "indirect":
                        nc.gpsimd.indirect_dma_start(
                            out=msg[:], out_offset=None,
                            in_=P2_full[:],
                            in_offset=bass.IndirectOffsetOnAxis(
                                ap=srcs_sb[:, o + q:o + q + 1], axis=0),
                        )
                    else:
                        r = (j * 128) % (NPAD - 128)
                        nc.sync.dma_start(out=msg[:], in_=P2_full[r:r + 128, :])
                    nc.tensor.matmul(out=agg[:], lhsT=selg[:, q * 128:(q + 1) * 128],
                                     rhs=msg[:],
                                     start=(q == 0), stop=(q == k - 1))
                z = wp.tile([128, FOUT], f32, tag="z2")
                nc.vector.tensor_scalar_mul(z[:], agg[:], dinv_sb[:, j:j + 1])
                nc.vector.tensor_tensor(out=z[:], in0=z[:], in1=b2_sb[:],
                                        op=mybir.AluOpType.add)
                za = wp.tile([128, FOUT], f32, tag="za2")
                nc.vector.tensor_scalar_mul(za[:], z[:], float(a_val))
                yo = wp.tile([128, FOUT], f32, tag="yo")
                nc.vector.tensor_tensor(out=yo[:], in0=z[:], in1=za[:],
                                        op=mybir.AluOpType.max)
                nc.sync.dma_start(out=out[j * 128:(j + 1) * 128, :], in_=yo[:])

    nc.compile()
    return nc


def _stage_inputs(x, W1, b1, W2, b2, dinv, srcs_dev, ldst_dev):
    x_pad = np.zeros((NPAD, FIN), TABLE_NP)
    x_pad[:N] = x
    in_maps = []
    W1d = W1.astype(TABLE_NP)
    W2d = W2.astype(TABLE_NP)
    b1d = np.broadcast_to(b1, (128, HID)).astype(np.float32).copy()
    b2d = np.broadcast_to(b2, (128, FOUT)).astype(np.float32).copy()
    iota_np = np.tile(np.arange(128, dtype=TABLE_NP), (128, 1)).copy()
    for c in range(NCORES):
        lo, hi = c * BPC, (c + 1) * BPC
        in_maps.append({
            "xT": np.ascontiguousarray(x_pad[lo:hi].T),
            "srcs": np.ascontiguousarray(srcs_dev[c]),
            "ldst": np.ascontiguousarray(ldst_dev[c]),
            "W1": W1d, "W2": W2d, "b1": b1d, "b2": b2d,
            "dinvb": np.ascontiguousarray(dinv[lo:hi].reshape(NBLK, 128).T),
            "iota": iota_np,
        })
    return in_maps


def kernel(x, edge_index, W1, b1, W2, b2, a, _want_results=False, _trace=False):
    x = np.asarray(x, np.float32)
    edge_index = np.asarray(edge_index, np.int32)
    dinv, srcs_dev, ldst_dev, Kj, C = _preprocess(edge_index)
    key = (Kj, float(a))
    if key not in _cache:
        _cache[key] = _build(Kj, C, float(a))
    nc = _cache[key]
    in_maps = _stage_inputs(x, np.asarray(W1, np.float32), np.asarray(b1, np.float32),
                            np.asarray(W2, np.float32), np.asarray(b2, np.float32),
                            dinv, srcs_dev, ldst_dev)
    res = run_bass_kernel_spmd(nc, in_maps, core_ids=list(range(NCORES)),
                               trace=_trace)
    outs = [res.results[c]["out"] for c in range(NCORES)]
    full = np.concatenate(outs, axis=0)[:N]
    if _want_results:
        return full.astype(np.float32), res
    return full.astype(np.float32)
